# revision 7
# baseline (speedup 1.0000x reference)
"""Trainium2 Bass kernel for nn_DynamicResolutionAttention.

B=2, T=2048, C=1024, H=16 heads, head_dim=64.
  q/k/v = x @ W{q,k,v}.T + b     (per-head views)
  attn  = softmax(q k^T / sqrt(hd) * (0.5 + 0.5*resolve))
  y     = attn @ v ; out = y @ Wp.T + bp
Sharding (8 cores): core c = (batch b=c//4, head-group hg=c%4, 4 heads each).

Structure (v2 — software-pipelined flat loop):
  The scalar engine runs ONLY the exp activations (the per-core floor:
  4 heads x T x T = 16.8M elems at 1 elem/cycle/lane) — every DMA issue
  lives on sync/vector/gpsimd/tensor queues so the ACT stream never
  queues behind anything.  Attention is one flat loop over 8
  pair-quarters x 8 k2-steps: per step S-matmuls (two heads row-tiled
  concurrently on the PE via tile_position auto-derivation), the two
  exps, then the PREVIOUS step's AV matmuls (one-step lag so the PE
  never waits on an in-flight exp).  QKV/V projections for later pairs
  are interleaved into the first two pair-quarters' steps so exp starts
  ~15us into the kernel instead of ~90us.  Softmax denominators come
  from a ones-column appended to V (k-major S^T layout, no on-chip
  transposes); per-pair both heads' denominators are batched into one
  [2,512] reciprocal.  y^T is AllGathered per token-quarter within each
  batch's 4 cores; the four output projections (column-split -> no
  all-reduce) run after the flat loop, each pipelining against the
  serialized AllGather chain.
"""

import sys

for _p in ("/opt/trn_rl_repo",):
    if _p not in sys.path:
        sys.path.insert(0, _p)

import numpy as np

B, T, C, H = 2, 2048, 1024, 16
HD = C // H            # 64
NCORES = 8
HL = 4                 # heads per core
NP = HL // 2           # head pairs per core
CL = HL * HD           # 256 local channels
CIN = C // 128         # 8 contraction tiles
KT_TILES = T // 128    # 16
QC = T // 512          # 4 query chunks

_prog_cache = {}


def _build_program():
    import concourse.mybir as mybir
    import concourse.tile as tile
    from concourse import bacc

    f32 = mybir.dt.float32
    bf16 = mybir.dt.bfloat16

    nc = bacc.Bacc("TRN2", target_bir_lowering=False, debug=False,
                   num_devices=NCORES)

    xP = nc.dram_tensor("xP", [128, CIN, T], bf16, kind="ExternalInput")
    wqP = nc.dram_tensor("wqP", [128, CIN, CL], bf16, kind="ExternalInput")
    wkP = nc.dram_tensor("wkP", [128, CIN, CL], bf16, kind="ExternalInput")
    wvP = nc.dram_tensor("wvP", [128, CIN, CL], bf16, kind="ExternalInput")
    wpP = nc.dram_tensor("wpP", [128, CIN, CL], bf16, kind="ExternalInput")
    bqC = nc.dram_tensor("bqC", [128, NP], f32, kind="ExternalInput")
    bkC = nc.dram_tensor("bkC", [128, NP], f32, kind="ExternalInput")
    bv = nc.dram_tensor("bv", [1, CL], bf16, kind="ExternalInput")
    bp = nc.dram_tensor("bp", [1, CL], bf16, kind="ExternalInput")
    rlv = nc.dram_tensor("rlv", [1, 1], f32, kind="ExternalInput")
    ones_d = nc.dram_tensor("ones_d", [1, 512], bf16, kind="ExternalInput")
    z = nc.dram_tensor("z", [T, CL], f32, kind="ExternalOutput")

    with tile.TileContext(nc) as tc:
        with tc.tile_pool(name="const", bufs=1) as const, \
             tc.tile_pool(name="big", bufs=1) as big, \
             tc.tile_pool(name="xp", bufs=2) as xp, \
             tc.tile_pool(name="work", bufs=3) as work, \
             tc.tile_pool(name="ps", bufs=2, space="PSUM") as ps, \
             tc.tile_pool(name="dram", bufs=1, space="DRAM") as dram:

            # ---- constant / weight / input loads (never on scalar) ----
            st = const.tile([128, 1], f32)
            nc.sync.dma_start(st[:], rlv[:].to_broadcast((128, 1)))
            nc.vector.tensor_scalar(st[:], st[:], 0.0625, 0.0625,
                                    mybir.AluOpType.mult, mybir.AluOpType.add)

            ones512 = const.tile([1, 512], bf16)
            nc.sync.dma_start(ones512[:], ones_d[:])
            ones128 = const.tile([1, 128], bf16)
            nc.sync.dma_start(ones128[:], ones_d[:, 0:128])

            bqC_sb = const.tile([128, NP], f32)
            bkC_sb = const.tile([128, NP], f32)
            bv_sb = const.tile([1, CL], bf16)
            bp_sb = const.tile([1, CL], bf16)
            nc.sync.dma_start(bqC_sb[:], bqC[:])
            nc.sync.dma_start(bkC_sb[:], bkC[:])
            nc.sync.dma_start(bv_sb[:], bv[:])
            nc.sync.dma_start(bp_sb[:], bp[:])

            # DMA queues: sync + gpsimd + (preload-only) scalar.  The
            # scalar queue issues a few loads at t=0 — all retired long
            # before its first ACT — and nothing else, so the exp
            # stream never queues behind a DMA.
            wq_sb = big.tile([128, CIN, CL], bf16)
            wk_sb = big.tile([128, CIN, CL], bf16)
            wv_sb = big.tile([128, CIN, CL], bf16)
            wp_sb = big.tile([128, CIN, CL], bf16)
            nc.sync.dma_start(wq_sb[:, 0:4, :], wqP[:, 0:4, :])
            nc.scalar.dma_start(wq_sb[:, 4:8, :], wqP[:, 4:8, :])
            nc.gpsimd.dma_start(wk_sb[:, 0:4, :], wkP[:, 0:4, :])
            nc.scalar.dma_start(wk_sb[:, 4:8, :], wkP[:, 4:8, :])

            # x resident: 4 token-quarter chunks spread over the queues
            xs = xp.tile([128, CIN, T], bf16, tag="xT")
            for tq, eng in enumerate((nc.sync, nc.scalar, nc.gpsimd,
                                      nc.gpsimd)):
                eng.dma_start(xs[:, :, tq * 512:(tq + 1) * 512],
                              xP[:, :, tq * 512:(tq + 1) * 512])
            nc.sync.dma_start(wv_sb[:, 0:4, :], wvP[:, 0:4, :])
            nc.scalar.dma_start(wv_sb[:, 4:8, :], wvP[:, 4:8, :])

            QTp = [big.tile([128, T], bf16, name=f"QT{p}") for p in range(NP)]
            KTp = [big.tile([128, T], bf16, name=f"KT{p}") for p in range(NP)]
            Vp = [big.tile([128, KT_TILES, 2, HD + 1], bf16, name=f"V{p}")
                  for p in range(NP)]
            for p in range(NP):
                nc.sync.dma_start(
                    Vp[p][:, :, :, HD].rearrange("p a b -> p (a b)"),
                    ones_d[0:1, 0:1].to_broadcast((128, KT_TILES * 2)))

            # ---- projection building blocks -------------------------
            def qk_block(pair, is_q, ch):
                w_sb = wq_sb if is_q else wk_sb
                OUT = (QTp if is_q else KTp)[pair]
                bc = bqC_sb if is_q else bkC_sb
                pc = slice(pair * 128, (pair + 1) * 128)
                pm = ps.tile([128, 2, 512], f32, tag="s", name="pm", bufs=2)
                pm = pm[:, 0, :]
                for ci in range(CIN):
                    nc.tensor.matmul(
                        pm, w_sb[:, ci, pc],
                        xs[:, ci, ch * 512:(ch + 1) * 512],
                        start=(ci == 0), stop=(ci == CIN - 1))
                dst = OUT[:, ch * 512:(ch + 1) * 512]
                if is_q:
                    # (q + bias) * softmax temperature
                    nc.vector.tensor_scalar(
                        dst, pm, bc[:, pair:pair + 1], st[:],
                        mybir.AluOpType.add, mybir.AluOpType.mult)
                else:
                    nc.vector.tensor_scalar_add(
                        dst, pm, bc[:, pair:pair + 1])

            def v_block(tt):
                pv = ps.tile([128, 2, 512], f32, tag="s", name="pv", bufs=2)
                pv = pv[:, 0, 0:CL]
                nc.tensor.matmul(pv, ones128[:], bv_sb[:],
                                 start=True, stop=False)
                for ci in range(CIN):
                    nc.tensor.matmul(
                        pv, xs[:, ci, tt * 128:(tt + 1) * 128],
                        wv_sb[:, ci, :],
                        start=False, stop=(ci == CIN - 1))
                for p in range(NP):
                    nc.vector.tensor_copy(
                        Vp[p][:, tt, :, 0:HD],
                        pv[:, p * 128:(p + 1) * 128]
                        .rearrange("p (h d) -> p h d", h=2))

            # ---- attention building blocks --------------------------
            ag_in = [dram.tile([CL, 512], bf16, name=f"ag_in{i}")
                     for i in range(QC)]
            rec_d = dram.tile([16, 512], f32, name="rec_d")
            ag_out = [dram.tile([4, CL, 512], bf16, name=f"ag_out{i}")
                      for i in range(QC)]

            pys_of = {}          # pq -> [py_h0, py_h1]
            pt_of = {}           # (pq, k2) -> pt tile

            def emit_S_ACT(pq, k2):
                qc, pair = divmod(pq, 2)
                QT_, KT_ = QTp[pair], KTp[pair]
                qs = slice(qc * 512, (qc + 1) * 512)
                pss = []
                for hh in range(2):
                    off = hh * HD
                    pt_s = ps.tile([128, 2, 512], f32, tag="s",
                                   name="pss", bufs=2)
                    for j in range(2):
                        kt = k2 * 2 + j
                        nc.tensor.matmul(
                            pt_s[:, j, :],
                            KT_[off:off + HD, kt * 128:(kt + 1) * 128],
                            QT_[off:off + HD, qs],
                            start=True, stop=True)
                    pss.append(pt_s)
                for hh in range(2):
                    pt = work.tile([128, 2, 512], bf16, tag="pt",
                                   name="pt", bufs=6)
                    nc.scalar.activation(
                        pt[:], pss[hh][:],
                        mybir.ActivationFunctionType.Exp)
                    pt_of[(pq, k2, hh)] = pt

            def emit_AV(pq, k2):
                qc, pair = divmod(pq, 2)
                V_ = Vp[pair]
                if k2 == 0:
                    pys_of[pq] = [ps.tile([HD + 1, 512], f32, tag="y",
                                          name=f"py{pq}_{i}", bufs=3)
                                  for i in range(2)]
                pys = pys_of[pq]
                for hh in range(2):
                    pt = pt_of.pop((pq, k2, hh))
                    for j in range(2):
                        kt = k2 * 2 + j
                        nc.tensor.matmul(
                            pys[hh][:], V_[:, kt, hh, :],
                            pt[:, j, :],
                            start=(kt == 0),
                            stop=(kt == KT_TILES - 1))

            def emit_norm(pq):
                # Both heads' denominators land on 32-aligned partitions
                # (DVE writes must start 32-aligned) so ONE reciprocal
                # instruction covers the pair (recip is 8 cyc/elem along
                # the free dim — partitions are free lanes).
                qc, pair = divmod(pq, 2)
                pys = pys_of.pop(pq)
                den = work.tile([33, 512], f32, tag="den", bufs=2)
                for hh in range(2):
                    nc.vector.tensor_copy(den[32 * hh:32 * hh + 1, :],
                                          pys[hh][HD:HD + 1, :])
                rcp = work.tile([33, 512], f32, tag="rcp", bufs=2)
                nc.vector.reciprocal(rcp[:], den[:])
                for hh in range(2):
                    nc.sync.dma_start(rec_d[2 * pq + hh:2 * pq + hh + 1, :],
                                      rcp[32 * hh:32 * hh + 1, :])
                for hh in range(2):
                    h = pair * 2 + hh
                    pbs = work.tile([HD, 512], f32, tag="pbs", bufs=4)
                    nc.sync.dma_start(
                        pbs[:],
                        rec_d[2 * pq + hh:2 * pq + hh + 1, :]
                        .to_broadcast((HD, 512)))
                    yt = work.tile([HD, 512], bf16, tag="yt", bufs=4)
                    nc.vector.tensor_mul(yt[:], pys[hh][0:HD, :], pbs[:])
                    nc.sync.dma_start(
                        ag_in[qc][h * HD:(h + 1) * HD, :], yt[:])

            def emit_AG(qc):
                nc.gpsimd.collective_compute(
                    "AllGather", mybir.AluOpType.bypass,
                    replica_groups=[[0, 1, 2, 3], [4, 5, 6, 7]],
                    ins=[ag_in[qc].opt()], outs=[ag_out[qc].opt()])

            # ---- phase 1: QKV projections for pair 0 ----------------
            # Q ch0 + all K first: the first S-matmuls need QT0[qc=0]
            # and KT0 progressively, so exp can start ~4 blocks in.
            qk_block(0, True, 0)
            for ch in range(QC):
                qk_block(0, False, ch)
            for ch in range(1, QC):
                qk_block(0, True, ch)

            # ---- flat software-pipelined attention loop -------------
            # pq = (query-quarter, head-pair); per step: S+exp for
            # (pq,k2), AV for the previous step.  pq0 interleaves the
            # V projection; pq1 interleaves pair-1's Q/K projections.
            prev = None
            for pq in range(2 * QC):
                qc, pair = divmod(pq, 2)
                for k2 in range(8):
                    emit_S_ACT(pq, k2)
                    if prev is not None:
                        emit_AV(*prev)
                        if prev[1] == 7:
                            ppq = prev[0]
                            emit_norm(ppq)
                            if ppq % 2 == 1:
                                emit_AG(ppq // 2)
                                if ppq == 1:
                                    # wp loads for the output projection
                                    nc.sync.dma_start(wp_sb[:, 0:4, :],
                                                      wpP[:, 0:4, :])
                                    nc.sync.dma_start(wp_sb[:, 4:8, :],
                                                      wpP[:, 4:8, :])
                    prev = (pq, k2)
                    if pq == 0:
                        v_block(2 * k2)
                        v_block(2 * k2 + 1)
                        if k2 == 7:
                            qk_block(1, True, 0)
                            qk_block(1, False, 0)
                    elif pq == 1:
                        if k2 < 3:
                            qk_block(1, False, k2 + 1)
                        elif k2 < 6:
                            qk_block(1, True, k2 - 2)
            emit_AV(*prev)
            emit_norm(prev[0])
            emit_AG(QC - 1)

            # ---- output projection (column-split, no all-reduce) ----
            # Each out-proj waits only on its own quarter's AllGather,
            # pipelining against the serialized collective chain.
            for qc in range(QC):
                ysb = xp.tile([128, CIN, 512], bf16, tag="xT", name="ysb")
                agf = ag_out[qc][:].rearrange("g c t -> (g c) t") \
                                   .rearrange("(o p) t -> p o t", p=128)
                nc.sync.dma_start(ysb[:, 0:4, :], agf[:, 0:4, :])
                nc.sync.dma_start(ysb[:, 4:8, :], agf[:, 4:8, :])
                for tt in range(4):
                    pz = ps.tile([128, 512], f32, tag="z", name="pz", bufs=1)
                    pz = pz[:, 0:CL]
                    nc.tensor.matmul(pz, ones128[:], bp_sb[:],
                                     start=True, stop=False)
                    for ci in range(CIN):
                        nc.tensor.matmul(
                            pz, ysb[:, ci, tt * 128:(tt + 1) * 128],
                            wp_sb[:, ci, :],
                            start=False, stop=(ci == CIN - 1))
                    zs = work.tile([128, CL], f32, tag="zs", bufs=2)
                    nc.vector.tensor_copy(zs[:], pz)
                    t0 = qc * 512 + tt * 128
                    nc.sync.dma_start(z[t0:t0 + 128, :], zs[:])

    nc.compile()
    return nc


def _get_program():
    if "nc" not in _prog_cache:
        _prog_cache["nc"] = _build_program()
    return _prog_cache["nc"]


def _pmajor(a2d):
    """[C, N] -> [128, C//128, N] partition-major contiguous."""
    Cdim, N = a2d.shape
    return np.ascontiguousarray(
        a2d.reshape(CIN, 128, N).transpose(1, 0, 2))


def kernel(x, Wq, bq, Wk, bk, Wv, bv, Wp, bp, resolve_level):
    import ml_dtypes
    from concourse.bass_utils import run_bass_kernel_spmd

    bfl = ml_dtypes.bfloat16
    nc = _get_program()

    x = np.asarray(x, np.float32)
    rl = np.asarray(resolve_level, np.float32).reshape(1, 1)

    xP_b = [_pmajor(np.ascontiguousarray(x[b].T).astype(bfl))
            for b in range(B)]
    in_maps = []
    for c in range(NCORES):
        b, hg = c // 4, c % 4
        cs = slice(hg * CL, (hg + 1) * CL)
        in_maps.append({
            "xP": xP_b[b],
            "wqP": _pmajor(np.asarray(Wq, np.float32)[cs, :].T.astype(bfl)),
            "wkP": _pmajor(np.asarray(Wk, np.float32)[cs, :].T.astype(bfl)),
            "wvP": _pmajor(np.asarray(Wv, np.float32)[cs, :].T.astype(bfl)),
            "wpP": _pmajor(np.asarray(Wp, np.float32)[cs, :].T.astype(bfl)),
            "bqC": np.ascontiguousarray(
                np.asarray(bq, np.float32)[cs].reshape(NP, 128).T),
            "bkC": np.ascontiguousarray(
                np.asarray(bk, np.float32)[cs].reshape(NP, 128).T),
            "bv": np.asarray(bv, np.float32)[cs].reshape(1, CL).astype(bfl),
            "bp": np.asarray(bp, np.float32)[cs].reshape(1, CL).astype(bfl),
            "rlv": rl,
            "ones_d": np.ones((1, 512), bfl),
        })

    res = run_bass_kernel_spmd(nc, in_maps, core_ids=list(range(NCORES)))

    out = np.empty((B, T, C), np.float32)
    for c in range(NCORES):
        b, hg = c // 4, c % 4
        out[b, :, hg * CL:(hg + 1) * CL] = res.results[c]["z"]
    return out


# revision 9
# speedup vs baseline: 1.1108x; 1.1108x over previous
"""Trainium2 Bass kernel for nn_DynamicResolutionAttention.

B=2, T=2048, C=1024, H=16 heads, head_dim=64.
  q/k/v = x @ W{q,k,v}.T + b     (per-head views)
  attn  = softmax(q k^T / sqrt(hd) * (0.5 + 0.5*resolve))
  y     = attn @ v ; out = y @ Wp.T + bp
Sharding (8 cores): core c = (batch b=c//4, head-group hg=c%4, 4 heads each).

Structure (v2 — software-pipelined flat loop):
  The scalar engine runs ONLY the exp activations (the per-core floor:
  4 heads x T x T = 16.8M elems at 1 elem/cycle/lane) — every DMA issue
  lives on sync/vector/gpsimd/tensor queues so the ACT stream never
  queues behind anything.  Attention is one flat loop over 8
  pair-quarters x 8 k2-steps: per step S-matmuls (two heads row-tiled
  concurrently on the PE via tile_position auto-derivation), the two
  exps, then the PREVIOUS step's AV matmuls (one-step lag so the PE
  never waits on an in-flight exp).  QKV/V projections for later pairs
  are interleaved into the first two pair-quarters' steps so exp starts
  ~15us into the kernel instead of ~90us.  Softmax denominators come
  from a ones-column appended to V (k-major S^T layout, no on-chip
  transposes); per-pair both heads' denominators are batched into one
  [2,512] reciprocal.  y^T is AllGathered per token-quarter within each
  batch's 4 cores; the four output projections (column-split -> no
  all-reduce) run after the flat loop, each pipelining against the
  serialized AllGather chain.
"""

import sys

for _p in ("/opt/trn_rl_repo",):
    if _p not in sys.path:
        sys.path.insert(0, _p)

import numpy as np

B, T, C, H = 2, 2048, 1024, 16
HD = C // H            # 64
NCORES = 8
HL = 4                 # heads per core
NP = HL // 2           # head pairs per core
CL = HL * HD           # 256 local channels
CIN = C // 128         # 8 contraction tiles
KT_TILES = T // 128    # 16
QC = T // 512          # 4 query chunks

_prog_cache = {}


def _build_program():
    import concourse.mybir as mybir
    import concourse.tile as tile
    from concourse import bacc

    f32 = mybir.dt.float32
    bf16 = mybir.dt.bfloat16

    nc = bacc.Bacc("TRN2", target_bir_lowering=False, debug=False,
                   num_devices=NCORES)

    xP = nc.dram_tensor("xP", [128, CIN, T], bf16, kind="ExternalInput")
    wqP = nc.dram_tensor("wqP", [128, CIN, CL], bf16, kind="ExternalInput")
    wkP = nc.dram_tensor("wkP", [128, CIN, CL], bf16, kind="ExternalInput")
    wvP = nc.dram_tensor("wvP", [128, CIN, CL], bf16, kind="ExternalInput")
    wpP = nc.dram_tensor("wpP", [128, CIN, CL], bf16, kind="ExternalInput")
    bqC = nc.dram_tensor("bqC", [128, NP], f32, kind="ExternalInput")
    bkC = nc.dram_tensor("bkC", [128, NP], f32, kind="ExternalInput")
    bv = nc.dram_tensor("bv", [1, CL], bf16, kind="ExternalInput")
    bp = nc.dram_tensor("bp", [1, CL], bf16, kind="ExternalInput")
    rlv = nc.dram_tensor("rlv", [1, 1], f32, kind="ExternalInput")
    ones_d = nc.dram_tensor("ones_d", [1, 512], bf16, kind="ExternalInput")
    z = nc.dram_tensor("z", [T, CL], f32, kind="ExternalOutput")

    with tile.TileContext(nc) as tc:
        with tc.tile_pool(name="const", bufs=1) as const, \
             tc.tile_pool(name="big", bufs=1) as big, \
             tc.tile_pool(name="xp", bufs=2) as xp, \
             tc.tile_pool(name="work", bufs=3) as work, \
             tc.tile_pool(name="ps", bufs=2, space="PSUM") as ps, \
             tc.tile_pool(name="dram", bufs=1, space="DRAM") as dram:

            # ---- constant / weight / input loads (never on scalar) ----
            st = const.tile([128, 1], f32)
            nc.sync.dma_start(st[:], rlv[:].to_broadcast((128, 1)))
            nc.vector.tensor_scalar(st[:], st[:], 0.0625, 0.0625,
                                    mybir.AluOpType.mult, mybir.AluOpType.add)

            ones512 = const.tile([1, 512], bf16)
            nc.sync.dma_start(ones512[:], ones_d[:])
            ones128 = const.tile([1, 128], bf16)
            nc.sync.dma_start(ones128[:], ones_d[:, 0:128])

            bqC_sb = const.tile([128, NP], f32)
            bkC_sb = const.tile([128, NP], f32)
            bv_sb = const.tile([1, CL], bf16)
            bp_sb = const.tile([1, CL], bf16)
            nc.sync.dma_start(bqC_sb[:], bqC[:])
            nc.sync.dma_start(bkC_sb[:], bkC[:])
            nc.sync.dma_start(bv_sb[:], bv[:])
            nc.sync.dma_start(bp_sb[:], bp[:])

            # DMA queues: sync + gpsimd + (preload-only) scalar.  The
            # scalar queue issues a few loads at t=0 — all retired long
            # before its first ACT — and nothing else, so the exp
            # stream never queues behind a DMA.
            wq_sb = big.tile([128, CIN, CL], bf16)
            wk_sb = big.tile([128, CIN, CL], bf16)
            wv_sb = big.tile([128, CIN, CL], bf16)
            wp_sb = big.tile([128, CIN, CL], bf16)
            nc.sync.dma_start(wq_sb[:, 0:4, :], wqP[:, 0:4, :])
            nc.scalar.dma_start(wq_sb[:, 4:8, :], wqP[:, 4:8, :])
            nc.gpsimd.dma_start(wk_sb[:, 0:4, :], wkP[:, 0:4, :])
            nc.scalar.dma_start(wk_sb[:, 4:8, :], wkP[:, 4:8, :])

            # x resident: 4 token-quarter chunks spread over the queues
            xs = xp.tile([128, CIN, T], bf16, tag="xT")
            for tq, eng in enumerate((nc.sync, nc.scalar, nc.gpsimd,
                                      nc.gpsimd)):
                eng.dma_start(xs[:, :, tq * 512:(tq + 1) * 512],
                              xP[:, :, tq * 512:(tq + 1) * 512])
            nc.sync.dma_start(wv_sb[:, 0:4, :], wvP[:, 0:4, :])
            nc.scalar.dma_start(wv_sb[:, 4:8, :], wvP[:, 4:8, :])

            QTp = [big.tile([128, T], bf16, name=f"QT{p}") for p in range(NP)]
            KTp = [big.tile([128, T], bf16, name=f"KT{p}") for p in range(NP)]
            Vp = [big.tile([128, KT_TILES, 2, HD + 1], bf16, name=f"V{p}")
                  for p in range(NP)]
            for p in range(NP):
                nc.sync.dma_start(
                    Vp[p][:, :, :, HD].rearrange("p a b -> p (a b)"),
                    ones_d[0:1, 0:1].to_broadcast((128, KT_TILES * 2)))

            # ---- projection building blocks -------------------------
            def qk_block(pair, is_q, ch):
                w_sb = wq_sb if is_q else wk_sb
                OUT = (QTp if is_q else KTp)[pair]
                bc = bqC_sb if is_q else bkC_sb
                pc = slice(pair * 128, (pair + 1) * 128)
                pm = ps.tile([128, 2, 512], f32, tag="s", name="pm", bufs=3)
                pm = pm[:, 0, :]
                for ci in range(CIN):
                    nc.tensor.matmul(
                        pm, w_sb[:, ci, pc],
                        xs[:, ci, ch * 512:(ch + 1) * 512],
                        start=(ci == 0), stop=(ci == CIN - 1))
                dst = OUT[:, ch * 512:(ch + 1) * 512]
                if is_q:
                    # (q + bias) * softmax temperature
                    nc.vector.tensor_scalar(
                        dst, pm, bc[:, pair:pair + 1], st[:],
                        mybir.AluOpType.add, mybir.AluOpType.mult)
                else:
                    nc.vector.tensor_scalar_add(
                        dst, pm, bc[:, pair:pair + 1])

            def v_block(tt):
                pv = ps.tile([128, 2, 512], f32, tag="s", name="pv", bufs=3)
                pv = pv[:, 0, 0:CL]
                nc.tensor.matmul(pv, ones128[:], bv_sb[:],
                                 start=True, stop=False)
                for ci in range(CIN):
                    nc.tensor.matmul(
                        pv, xs[:, ci, tt * 128:(tt + 1) * 128],
                        wv_sb[:, ci, :],
                        start=False, stop=(ci == CIN - 1))
                for p in range(NP):
                    nc.vector.tensor_copy(
                        Vp[p][:, tt, :, 0:HD],
                        pv[:, p * 128:(p + 1) * 128]
                        .rearrange("p (h d) -> p h d", h=2))

            # ---- attention building blocks --------------------------
            ag_in = [dram.tile([CL, 512], bf16, name=f"ag_in{i}")
                     for i in range(QC)]
            rec_d = dram.tile([16, 512], f32, name="rec_d")
            ag_out = [dram.tile([4, CL, 512], bf16, name=f"ag_out{i}")
                      for i in range(QC)]

            pys_of = {}          # pq -> [py_h0, py_h1]
            pt_of = {}           # (pq, k2) -> pt tile

            def emit_S_ACT(pq, k2):
                qc, pair = divmod(pq, 2)
                QT_, KT_ = QTp[pair], KTp[pair]
                qs = slice(qc * 512, (qc + 1) * 512)
                pss = []
                for hh in range(2):
                    off = hh * HD
                    pt_s = ps.tile([128, 2, 512], f32, tag="s",
                                   name="pss", bufs=3)
                    for j in range(2):
                        kt = k2 * 2 + j
                        nc.tensor.matmul(
                            pt_s[:, j, :],
                            KT_[off:off + HD, kt * 128:(kt + 1) * 128],
                            QT_[off:off + HD, qs],
                            start=True, stop=True)
                    pss.append(pt_s)
                for hh in range(2):
                    pt = work.tile([128, 2, 512], bf16, tag="pt",
                                   name="pt", bufs=6)
                    nc.scalar.activation(
                        pt[:], pss[hh][:],
                        mybir.ActivationFunctionType.Exp)
                    pt_of[(pq, k2, hh)] = pt

            def emit_AV(pq, k2):
                qc, pair = divmod(pq, 2)
                V_ = Vp[pair]
                if k2 == 0:
                    pys_of[pq] = [ps.tile([HD + 1, 512], f32, tag="y",
                                          name=f"py{pq}_{i}", bufs=2)
                                  for i in range(2)]
                pys = pys_of[pq]
                for hh in range(2):
                    pt = pt_of.pop((pq, k2, hh))
                    for j in range(2):
                        kt = k2 * 2 + j
                        nc.tensor.matmul(
                            pys[hh][:], V_[:, kt, hh, :],
                            pt[:, j, :],
                            start=(kt == 0),
                            stop=(kt == KT_TILES - 1))

            def emit_norm(pq):
                # Both heads' denominators land on 32-aligned partitions
                # (DVE writes must start 32-aligned) so ONE reciprocal
                # instruction covers the pair (recip is 8 cyc/elem along
                # the free dim — partitions are free lanes).
                qc, pair = divmod(pq, 2)
                pys = pys_of.pop(pq)
                den = work.tile([33, 512], f32, tag="den", bufs=2)
                for hh in range(2):
                    nc.vector.tensor_copy(den[32 * hh:32 * hh + 1, :],
                                          pys[hh][HD:HD + 1, :])
                rcp = work.tile([33, 512], f32, tag="rcp", bufs=2)
                nc.vector.reciprocal(rcp[:], den[:])
                for hh in range(2):
                    nc.sync.dma_start(rec_d[2 * pq + hh:2 * pq + hh + 1, :],
                                      rcp[32 * hh:32 * hh + 1, :])
                for hh in range(2):
                    h = pair * 2 + hh
                    pbs = work.tile([HD, 512], f32, tag="pbs", bufs=4)
                    nc.sync.dma_start(
                        pbs[:],
                        rec_d[2 * pq + hh:2 * pq + hh + 1, :]
                        .to_broadcast((HD, 512)))
                    yt = work.tile([HD, 512], bf16, tag="yt", bufs=4)
                    nc.vector.tensor_mul(yt[:], pys[hh][0:HD, :], pbs[:])
                    nc.sync.dma_start(
                        ag_in[qc][h * HD:(h + 1) * HD, :], yt[:])

            def emit_AG(qc):
                nc.gpsimd.collective_compute(
                    "AllGather", mybir.AluOpType.bypass,
                    replica_groups=[[0, 1, 2, 3], [4, 5, 6, 7]],
                    ins=[ag_in[qc].opt()], outs=[ag_out[qc].opt()])

            # ---- phase 1: QKV projections (dense — keeps the PE HAM
            # clock warm; the exp stream is not the binding engine at
            # the observed throttled PE clock) ------------------------
            qk_block(0, True, 0)
            for ch in range(QC):
                qk_block(0, False, ch)
            for ch in range(1, QC):
                qk_block(0, True, ch)
            for tt in range(KT_TILES):
                v_block(tt)
            qk_block(1, True, 0)
            for ch in range(QC):
                qk_block(1, False, ch)
            for ch in range(1, QC):
                qk_block(1, True, ch)

            # ---- flat software-pipelined attention loop -------------
            # pq = (query-quarter, head-pair); per step: S+exp for
            # (pq,k2), then the PREVIOUS step's AV (one-step lag so the
            # PE never sits on an in-flight exp).
            prev = None
            for pq in range(2 * QC):
                for k2 in range(8):
                    emit_S_ACT(pq, k2)
                    if prev is not None:
                        emit_AV(*prev)
                        if prev[1] == 7:
                            ppq = prev[0]
                            emit_norm(ppq)
                            if ppq % 2 == 1:
                                emit_AG(ppq // 2)
                                if ppq == 1:
                                    # wp loads for the output projection
                                    nc.sync.dma_start(wp_sb[:, 0:4, :],
                                                      wpP[:, 0:4, :])
                                    nc.sync.dma_start(wp_sb[:, 4:8, :],
                                                      wpP[:, 4:8, :])
                    prev = (pq, k2)
            emit_AV(*prev)
            emit_norm(prev[0])
            emit_AG(QC - 1)

            # ---- output projection (column-split, no all-reduce) ----
            # Each out-proj waits only on its own quarter's AllGather,
            # pipelining against the serialized collective chain.
            for qc in range(QC):
              with tc.tile_wait_until(0.6 + 0.02 * qc):
                ysb = xp.tile([128, CIN, 512], bf16, tag="xT", name="ysb")
                agf = ag_out[qc][:].rearrange("g c t -> (g c) t") \
                                   .rearrange("(o p) t -> p o t", p=128)
                nc.sync.dma_start(ysb[:, 0:4, :], agf[:, 0:4, :])
                nc.sync.dma_start(ysb[:, 4:8, :], agf[:, 4:8, :])
                for tt in range(4):
                    pz = ps.tile([128, 2, 512], f32, tag="s", name="pz",
                                 bufs=3)
                    pz = pz[:, 0, 0:CL]
                    nc.tensor.matmul(pz, ones128[:], bp_sb[:],
                                     start=True, stop=False)
                    for ci in range(CIN):
                        nc.tensor.matmul(
                            pz, ysb[:, ci, tt * 128:(tt + 1) * 128],
                            wp_sb[:, ci, :],
                            start=False, stop=(ci == CIN - 1))
                    zs = work.tile([128, CL], f32, tag="zs", bufs=2)
                    nc.vector.tensor_copy(zs[:], pz)
                    t0 = qc * 512 + tt * 128
                    nc.sync.dma_start(z[t0:t0 + 128, :], zs[:])

    nc.compile()
    return nc


def _get_program():
    if "nc" not in _prog_cache:
        _prog_cache["nc"] = _build_program()
    return _prog_cache["nc"]


def _pmajor(a2d):
    """[C, N] -> [128, C//128, N] partition-major contiguous."""
    Cdim, N = a2d.shape
    return np.ascontiguousarray(
        a2d.reshape(CIN, 128, N).transpose(1, 0, 2))


def kernel(x, Wq, bq, Wk, bk, Wv, bv, Wp, bp, resolve_level):
    import ml_dtypes
    from concourse.bass_utils import run_bass_kernel_spmd

    bfl = ml_dtypes.bfloat16
    nc = _get_program()

    x = np.asarray(x, np.float32)
    rl = np.asarray(resolve_level, np.float32).reshape(1, 1)

    xP_b = [_pmajor(np.ascontiguousarray(x[b].T).astype(bfl))
            for b in range(B)]
    in_maps = []
    for c in range(NCORES):
        b, hg = c // 4, c % 4
        cs = slice(hg * CL, (hg + 1) * CL)
        in_maps.append({
            "xP": xP_b[b],
            "wqP": _pmajor(np.asarray(Wq, np.float32)[cs, :].T.astype(bfl)),
            "wkP": _pmajor(np.asarray(Wk, np.float32)[cs, :].T.astype(bfl)),
            "wvP": _pmajor(np.asarray(Wv, np.float32)[cs, :].T.astype(bfl)),
            "wpP": _pmajor(np.asarray(Wp, np.float32)[cs, :].T.astype(bfl)),
            "bqC": np.ascontiguousarray(
                np.asarray(bq, np.float32)[cs].reshape(NP, 128).T),
            "bkC": np.ascontiguousarray(
                np.asarray(bk, np.float32)[cs].reshape(NP, 128).T),
            "bv": np.asarray(bv, np.float32)[cs].reshape(1, CL).astype(bfl),
            "bp": np.asarray(bp, np.float32)[cs].reshape(1, CL).astype(bfl),
            "rlv": rl,
            "ones_d": np.ones((1, 512), bfl),
        })

    res = run_bass_kernel_spmd(nc, in_maps, core_ids=list(range(NCORES)))

    out = np.empty((B, T, C), np.float32)
    for c in range(NCORES):
        b, hg = c // 4, c % 4
        out[b, :, hg * CL:(hg + 1) * CL] = res.results[c]["z"]
    return out


# revision 10
# speedup vs baseline: 1.2788x; 1.1512x over previous
"""Trainium2 Bass kernel for nn_DynamicResolutionAttention.

B=2, T=2048, C=1024, H=16 heads, head_dim=64.
  q/k/v = x @ W{q,k,v}.T + b     (per-head views)
  attn  = softmax(q k^T / sqrt(hd) * (0.5 + 0.5*resolve))
  y     = attn @ v ; out = y @ Wp.T + bp
Sharding (8 cores): core c = (batch b=c//4, head-group hg=c%4, 4 heads each).

Structure (v2 — software-pipelined flat loop):
  The scalar engine runs ONLY the exp activations (the per-core floor:
  4 heads x T x T = 16.8M elems at 1 elem/cycle/lane) — every DMA issue
  lives on sync/vector/gpsimd/tensor queues so the ACT stream never
  queues behind anything.  Attention is one flat loop over 8
  pair-quarters x 8 k2-steps: per step S-matmuls (two heads row-tiled
  concurrently on the PE via tile_position auto-derivation), the two
  exps, then the PREVIOUS step's AV matmuls (one-step lag so the PE
  never waits on an in-flight exp).  QKV/V projections for later pairs
  are interleaved into the first two pair-quarters' steps so exp starts
  ~15us into the kernel instead of ~90us.  Softmax denominators come
  from a ones-column appended to V (k-major S^T layout, no on-chip
  transposes); per-pair both heads' denominators are batched into one
  [2,512] reciprocal.  y^T is AllGathered per token-quarter within each
  batch's 4 cores; the four output projections (column-split -> no
  all-reduce) run after the flat loop, each pipelining against the
  serialized AllGather chain.
"""

import sys

for _p in ("/opt/trn_rl_repo",):
    if _p not in sys.path:
        sys.path.insert(0, _p)

import numpy as np

B, T, C, H = 2, 2048, 1024, 16
HD = C // H            # 64
NCORES = 8
HL = 4                 # heads per core
NP = HL // 2           # head pairs per core
CL = HL * HD           # 256 local channels
CIN = C // 128         # 8 contraction tiles
KT_TILES = T // 128    # 16
QC = T // 512          # 4 query chunks

_prog_cache = {}


def _build_program():
    import concourse.mybir as mybir
    import concourse.tile as tile
    from concourse import bacc

    f32 = mybir.dt.float32
    bf16 = mybir.dt.bfloat16

    nc = bacc.Bacc("TRN2", target_bir_lowering=False, debug=False,
                   num_devices=NCORES)

    xP = nc.dram_tensor("xP", [128, CIN, T], bf16, kind="ExternalInput")
    wqP = nc.dram_tensor("wqP", [128, CIN, CL], bf16, kind="ExternalInput")
    wkP = nc.dram_tensor("wkP", [128, CIN, CL], bf16, kind="ExternalInput")
    wvP = nc.dram_tensor("wvP", [128, CIN, CL], bf16, kind="ExternalInput")
    wpP = nc.dram_tensor("wpP", [128, CIN, CL], bf16, kind="ExternalInput")
    bqC = nc.dram_tensor("bqC", [128, NP], f32, kind="ExternalInput")
    bkC = nc.dram_tensor("bkC", [128, NP], f32, kind="ExternalInput")
    bv = nc.dram_tensor("bv", [1, CL], bf16, kind="ExternalInput")
    bp = nc.dram_tensor("bp", [1, CL], bf16, kind="ExternalInput")
    rlv = nc.dram_tensor("rlv", [1, 1], f32, kind="ExternalInput")
    ones_d = nc.dram_tensor("ones_d", [1, 512], bf16, kind="ExternalInput")
    z = nc.dram_tensor("z", [T, CL], f32, kind="ExternalOutput")

    with tile.TileContext(nc) as tc:
        with tc.tile_pool(name="const", bufs=1) as const, \
             tc.tile_pool(name="big", bufs=1) as big, \
             tc.tile_pool(name="xp", bufs=2) as xp, \
             tc.tile_pool(name="work", bufs=3) as work, \
             tc.tile_pool(name="ps", bufs=2, space="PSUM") as ps, \
             tc.tile_pool(name="dram", bufs=1, space="DRAM") as dram:

            # ---- constant / weight / input loads (never on scalar) ----
            st = const.tile([128, 1], f32)
            nc.sync.dma_start(st[:], rlv[:].to_broadcast((128, 1)))
            nc.vector.tensor_scalar(st[:], st[:], 0.0625, 0.0625,
                                    mybir.AluOpType.mult, mybir.AluOpType.add)

            ones128 = const.tile([1, 128], bf16)
            nc.sync.dma_start(ones128[:], ones_d[:, 0:128])

            bqC_sb = const.tile([128, NP], f32)
            bkC_sb = const.tile([128, NP], f32)
            bv_sb = const.tile([1, CL], bf16)
            bp_sb = const.tile([1, CL], bf16)
            nc.sync.dma_start(bqC_sb[:], bqC[:])
            nc.sync.dma_start(bkC_sb[:], bkC[:])
            nc.sync.dma_start(bv_sb[:], bv[:])
            nc.sync.dma_start(bp_sb[:], bp[:])

            # DMA queues: sync + gpsimd + (preload-only) scalar.  The
            # scalar queue issues a few loads at t=0 — all retired long
            # before its first ACT — and nothing else, so the exp
            # stream never queues behind a DMA.
            wq_sb = big.tile([128, CIN, CL], bf16)
            wk_sb = big.tile([128, CIN, CL], bf16)
            wv_sb = big.tile([128, CIN, CL], bf16)
            wp_sb = big.tile([128, CIN, CL], bf16)
            nc.sync.dma_start(wq_sb[:, 0:4, :], wqP[:, 0:4, :])
            nc.scalar.dma_start(wq_sb[:, 4:8, :], wqP[:, 4:8, :])
            nc.gpsimd.dma_start(wk_sb[:, 0:4, :], wkP[:, 0:4, :])
            nc.scalar.dma_start(wk_sb[:, 4:8, :], wkP[:, 4:8, :])

            # x resident: 4 token-quarter chunks spread over the queues
            xs = xp.tile([128, CIN, T], bf16, tag="xT")
            for tq, eng in enumerate((nc.sync, nc.scalar, nc.gpsimd,
                                      nc.gpsimd)):
                eng.dma_start(xs[:, :, tq * 512:(tq + 1) * 512],
                              xP[:, :, tq * 512:(tq + 1) * 512])
            nc.sync.dma_start(wv_sb[:, 0:4, :], wvP[:, 0:4, :])
            nc.scalar.dma_start(wv_sb[:, 4:8, :], wvP[:, 4:8, :])

            QTp = [big.tile([128, T], bf16, name=f"QT{p}") for p in range(NP)]
            KTp = [big.tile([128, T], bf16, name=f"KT{p}") for p in range(NP)]
            Vp = [big.tile([128, KT_TILES, 2, HD + 1], bf16, name=f"V{p}")
                  for p in range(NP)]
            for p in range(NP):
                nc.sync.dma_start(
                    Vp[p][:, :, :, HD].rearrange("p a b -> p (a b)"),
                    ones_d[0:1, 0:1].to_broadcast((128, KT_TILES * 2)))

            # ---- projection building blocks -------------------------
            def qk_block(pair, is_q, ch):
                w_sb = wq_sb if is_q else wk_sb
                OUT = (QTp if is_q else KTp)[pair]
                bc = bqC_sb if is_q else bkC_sb
                pc = slice(pair * 128, (pair + 1) * 128)
                pm = ps.tile([128, 2, 512], f32, tag="s", name="pm", bufs=3)
                pm = pm[:, 0, :]
                for ci in range(CIN):
                    nc.tensor.matmul(
                        pm, w_sb[:, ci, pc],
                        xs[:, ci, ch * 512:(ch + 1) * 512],
                        start=(ci == 0), stop=(ci == CIN - 1))
                dst = OUT[:, ch * 512:(ch + 1) * 512]
                if is_q:
                    # (q + bias) * softmax temperature
                    nc.vector.tensor_scalar(
                        dst, pm, bc[:, pair:pair + 1], st[:],
                        mybir.AluOpType.add, mybir.AluOpType.mult)
                else:
                    nc.vector.tensor_scalar_add(
                        dst, pm, bc[:, pair:pair + 1])

            def v_block(tt):
                pv = ps.tile([128, 2, 512], f32, tag="s", name="pv", bufs=3)
                pv = pv[:, 0, 0:CL]
                nc.tensor.matmul(pv, ones128[:], bv_sb[:],
                                 start=True, stop=False)
                for ci in range(CIN):
                    nc.tensor.matmul(
                        pv, xs[:, ci, tt * 128:(tt + 1) * 128],
                        wv_sb[:, ci, :],
                        start=False, stop=(ci == CIN - 1))
                for p in range(NP):
                    nc.vector.tensor_copy(
                        Vp[p][:, tt, :, 0:HD],
                        pv[:, p * 128:(p + 1) * 128]
                        .rearrange("p (h d) -> p h d", h=2))

            # ---- attention building blocks --------------------------
            ag_in = [dram.tile([CL, 512], bf16, name=f"ag_in{i}")
                     for i in range(QC)]
            rec_d = dram.tile([16, 512], f32, name="rec_d")
            ag_out = [dram.tile([4, CL, 512], bf16, name=f"ag_out{i}")
                      for i in range(QC)]

            pys_of = {}          # pq -> [py_h0, py_h1]
            pt_of = {}           # (pq, k2) -> pt tile
            gates = [const.tile([1, 128], bf16, name=f"gate{i}")
                     for i in range(QC)]

            def emit_S_ACT(pq, k2):
                qc, pair = divmod(pq, 2)
                QT_, KT_ = QTp[pair], KTp[pair]
                qs = slice(qc * 512, (qc + 1) * 512)
                pss = []
                for hh in range(2):
                    off = hh * HD
                    pt_s = ps.tile([128, 2, 512], f32, tag="s",
                                   name="pss", bufs=3)
                    for j in range(2):
                        kt = k2 * 2 + j
                        nc.tensor.matmul(
                            pt_s[:, j, :],
                            KT_[off:off + HD, kt * 128:(kt + 1) * 128],
                            QT_[off:off + HD, qs],
                            start=True, stop=True)
                    pss.append(pt_s)
                for hh in range(2):
                    pt = work.tile([128, 2, 512], bf16, tag="pt",
                                   name="pt", bufs=10)
                    nc.scalar.activation(
                        pt[:], pss[hh][:],
                        mybir.ActivationFunctionType.Exp)
                    pt_of[(pq, k2, hh)] = pt

            def emit_AV(pq, k2):
                qc, pair = divmod(pq, 2)
                V_ = Vp[pair]
                if k2 == 0:
                    pys_of[pq] = [ps.tile([HD + 1, 512], f32, tag="y",
                                          name=f"py{pq}_{i}", bufs=2)
                                  for i in range(2)]
                pys = pys_of[pq]
                for hh in range(2):
                    pt = pt_of.pop((pq, k2, hh))
                    for j in range(2):
                        kt = k2 * 2 + j
                        nc.tensor.matmul(
                            pys[hh][:], V_[:, kt, hh, :],
                            pt[:, j, :],
                            start=(kt == 0),
                            stop=(kt == KT_TILES - 1))

            def emit_norm(pq):
                # Evacuate both accumulators PSUM->SBUF immediately (the
                # next pair-quarter's AV matmuls wait on these y slots),
                # then normalize from the SBUF copies off the critical
                # path.  Both heads' denominators land on 32-aligned
                # partitions (DVE writes must start 32-aligned) so ONE
                # reciprocal covers the pair (recip is 8 cyc/elem along
                # the free dim — partitions are free lanes).
                qc, pair = divmod(pq, 2)
                pys = pys_of.pop(pq)
                ycs = []
                for hh in range(2):
                    yc = work.tile([HD + 1, 512], f32, tag="yc", bufs=4)
                    nc.vector.tensor_copy(yc[:], pys[hh][:])
                    ycs.append(yc)
                den = work.tile([33, 512], f32, tag="den", bufs=2)
                for hh in range(2):
                    nc.vector.tensor_copy(den[32 * hh:32 * hh + 1, :],
                                          ycs[hh][HD:HD + 1, :])
                rcp = work.tile([33, 512], f32, tag="rcp", bufs=2)
                nc.vector.reciprocal(rcp[:], den[:])
                for hh in range(2):
                    nc.sync.dma_start(rec_d[2 * pq + hh:2 * pq + hh + 1, :],
                                      rcp[32 * hh:32 * hh + 1, :])
                for hh in range(2):
                    h = pair * 2 + hh
                    pbs = work.tile([HD, 512], f32, tag="pbs", bufs=4)
                    nc.sync.dma_start(
                        pbs[:],
                        rec_d[2 * pq + hh:2 * pq + hh + 1, :]
                        .to_broadcast((HD, 512)))
                    yt = work.tile([HD, 512], bf16, tag="yt", bufs=4)
                    nc.vector.tensor_mul(yt[:], ycs[hh][0:HD, :], pbs[:])
                    nc.sync.dma_start(
                        ag_in[qc][h * HD:(h + 1) * HD, :], yt[:])
                if pq >= 3 and pq % 2 == 1:
                    # scheduling gates: out-proj qc' may enter the engine
                    # queues only after quarter (pq-1)/2 is fully done,
                    # so a late AllGather can never head-block attention
                    g = (pq - 3) // 2
                    nc.vector.tensor_copy(gates[g][:], ones128[:])
                    if pq == 7:
                        nc.vector.tensor_copy(gates[3][:], ones128[:])

            def emit_AG(qc):
                nc.gpsimd.collective_compute(
                    "AllGather", mybir.AluOpType.bypass,
                    replica_groups=[[0, 1, 2, 3], [4, 5, 6, 7]],
                    ins=[ag_in[qc].opt()], outs=[ag_out[qc].opt()])

            # ---- phase 1: QKV projections (dense — keeps the PE HAM
            # clock warm; the exp stream is not the binding engine at
            # the observed throttled PE clock) ------------------------
            qk_block(0, True, 0)
            qk_block(0, False, 0)
            qk_block(0, False, 1)
            # exp warm-up: first four S+exp steps of (quarter 0, pair 0)
            # only need QT0[ch0] and KT0[ch0..1] — start the scalar
            # stream ~45us earlier than a monolithic projection phase
            for k2 in range(4):
                emit_S_ACT(0, k2)
            qk_block(0, False, 2)
            qk_block(0, False, 3)
            for ch in range(1, QC):
                qk_block(0, True, ch)
            for tt in range(KT_TILES):
                v_block(tt)
            qk_block(1, True, 0)
            for ch in range(QC):
                qk_block(1, False, ch)
            for ch in range(1, QC):
                qk_block(1, True, ch)

            # ---- flat software-pipelined attention loop -------------
            # pq = (query-quarter, head-pair); per step: S+exp for
            # (pq,k2), then the PREVIOUS step's AV (one-step lag so the
            # PE never sits on an in-flight exp).
            from collections import deque
            pending = deque((0, k2) for k2 in range(4))

            def drain_AV(n):
                for _ in range(n):
                    if not pending:
                        return
                    ppq, pk2 = pending.popleft()
                    emit_AV(ppq, pk2)
                    if pk2 == 7:
                        emit_norm(ppq)
                        if ppq % 2 == 1:
                            emit_AG(ppq // 2)
                            if ppq == 1:
                                # wp loads for the output projection
                                nc.sync.dma_start(wp_sb[:, 0:4, :],
                                                  wpP[:, 0:4, :])
                                nc.sync.dma_start(wp_sb[:, 4:8, :],
                                                  wpP[:, 4:8, :])

            for pq in range(2 * QC):
                for k2 in range(8):
                    if pq == 0 and k2 < 4:
                        continue       # hoisted into the projection phase
                    emit_S_ACT(pq, k2)
                    drain_AV(2 if len(pending) > 1 else 1)
                    pending.append((pq, k2))
            drain_AV(len(pending))

            # ---- output projection (column-split, no all-reduce) ----
            # Each out-proj waits only on its own quarter's AllGather,
            # pipelining against the serialized collective chain.
            for qc in range(QC):
                ysb = xp.tile([128, CIN, 512], bf16, tag="xT", name="ysb")
                agf = ag_out[qc][:].rearrange("g c t -> (g c) t") \
                                   .rearrange("(o p) t -> p o t", p=128)
                nc.sync.dma_start(ysb[:, 0:4, :], agf[:, 0:4, :])
                nc.sync.dma_start(ysb[:, 4:8, :], agf[:, 4:8, :])
                for tt in range(4):
                    pz = ps.tile([128, 2, 512], f32, tag="s", name="pz",
                                 bufs=3)
                    pz = pz[:, 0, 0:CL]
                    nc.tensor.matmul(pz, gates[qc][:], bp_sb[:],
                                     start=True, stop=False)
                    for ci in range(CIN):
                        nc.tensor.matmul(
                            pz, ysb[:, ci, tt * 128:(tt + 1) * 128],
                            wp_sb[:, ci, :],
                            start=False, stop=(ci == CIN - 1))
                    zs = work.tile([128, CL], f32, tag="zs", bufs=2)
                    nc.vector.tensor_copy(zs[:], pz)
                    t0 = qc * 512 + tt * 128
                    nc.sync.dma_start(z[t0:t0 + 128, :], zs[:])

    nc.compile()
    return nc


def _get_program():
    if "nc" not in _prog_cache:
        _prog_cache["nc"] = _build_program()
    return _prog_cache["nc"]


def _pmajor(a2d):
    """[C, N] -> [128, C//128, N] partition-major contiguous."""
    Cdim, N = a2d.shape
    return np.ascontiguousarray(
        a2d.reshape(CIN, 128, N).transpose(1, 0, 2))


def kernel(x, Wq, bq, Wk, bk, Wv, bv, Wp, bp, resolve_level):
    import ml_dtypes
    from concourse.bass_utils import run_bass_kernel_spmd

    bfl = ml_dtypes.bfloat16
    nc = _get_program()

    x = np.asarray(x, np.float32)
    rl = np.asarray(resolve_level, np.float32).reshape(1, 1)

    xP_b = [_pmajor(np.ascontiguousarray(x[b].T).astype(bfl))
            for b in range(B)]
    in_maps = []
    for c in range(NCORES):
        b, hg = c // 4, c % 4
        cs = slice(hg * CL, (hg + 1) * CL)
        in_maps.append({
            "xP": xP_b[b],
            "wqP": _pmajor(np.asarray(Wq, np.float32)[cs, :].T.astype(bfl)),
            "wkP": _pmajor(np.asarray(Wk, np.float32)[cs, :].T.astype(bfl)),
            "wvP": _pmajor(np.asarray(Wv, np.float32)[cs, :].T.astype(bfl)),
            "wpP": _pmajor(np.asarray(Wp, np.float32)[cs, :].T.astype(bfl)),
            "bqC": np.ascontiguousarray(
                np.asarray(bq, np.float32)[cs].reshape(NP, 128).T),
            "bkC": np.ascontiguousarray(
                np.asarray(bk, np.float32)[cs].reshape(NP, 128).T),
            "bv": np.asarray(bv, np.float32)[cs].reshape(1, CL).astype(bfl),
            "bp": np.asarray(bp, np.float32)[cs].reshape(1, CL).astype(bfl),
            "rlv": rl,
            "ones_d": np.ones((1, 512), bfl),
        })

    res = run_bass_kernel_spmd(nc, in_maps, core_ids=list(range(NCORES)))

    out = np.empty((B, T, C), np.float32)
    for c in range(NCORES):
        b, hg = c // 4, c % 4
        out[b, :, hg * CL:(hg + 1) * CL] = res.results[c]["z"]
    return out
